# revision 16
# baseline (speedup 1.0000x reference)
"""Trainium2 Bass kernel for nn_CreateOverlappingWindows.

out[b, t, w*C + c] = x_padded[b, t + w, c]  (SAME zero padding, n_context=9)

Flattening (w, c) -> 494 contiguous values, each output row is a contiguous
494-element window of the zero-padded flattened input:
    out[b, t, :] = xpad_flat[b, t*C : t*C + W*C]

Strategy (memory-regime): bf16 end-to-end (tolerance 2e-2; bf16 keeps f32's
exponent range so rel err <= 2^-9 ~ 2e-3 everywhere).

The output write dominates (19x the input bytes). Small descriptors are
descriptor-rate-bound (HWDGE ~10.4ns/desc; 988 B window descriptors cap at
~95 GB/s per ring), so instead:
  * SWDGE-load the tiny padded input into SBUF: per batch, partition p of
    100 holds the 988-element slice xpad_flat[b, 520*p : 520*p + 988]
    (20 output rows + 468-element halo).
  * DVE expands windows in-SBUF into a dense tile (int32-viewed copies,
    ~2.7us/batch; overlapping reads are free on a compute engine).
  * Two fused SWDGE stores (2 batches each, 200 x 19.8 KB fully-contiguous
    descriptors) stream the dense tile out at ~300+ GB/s. 19.8 KB is the
    per-descriptor sweet spot (>~20 KB halves the rate); big fused DMAs
    amortize the ~0.45us/engine doorbell ramp.

Sharding: pure data parallel - batch 32 split 4-per-core across 8 cores.
"""

import sys

sys.path.insert(0, "/opt/trn_rl_repo")

import ml_dtypes
import numpy as np
from concourse import bass, mybir
from concourse.ap import AP
from concourse.bass_utils import run_bass_kernel_spmd

_BF16 = mybir.dt.bfloat16
_I32 = mybir.dt.int32
_NPBF16 = ml_dtypes.bfloat16

_NCORES = 8
_B, _T, _C = 32, 2000, 26
_NCTX = 9
_W = 2 * _NCTX + 1  # 19
_WC = _W * _C  # 494
_PAD = _NCTX * _C  # 234
_BPC = _B // _NCORES  # 4 batches per core
_NP = _T * _C + 2 * _PAD  # 52468 padded flat length per batch
_TWC = _T * _WC  # 988000

_PPB = 100  # partitions per batch
_RPP = 20  # output rows per partition  (100 * 20 = 2000)
_STEP = _RPP * _C  # 520: flat-input stride between partition slices
_SEG = _STEP + (_WC - _C)  # 988: slice length incl. 468-element halo
_FI = _BPC * _SEG  # free elems/partition, input tile
_RW = _RPP * _WC  # 9880: dense output elems/partition/batch
_FO = _BPC * _RW  # 39520: free elems/partition, output tile

_nc_cache = None


def _build():
    global _nc_cache
    if _nc_cache is not None:
        return _nc_cache
    nc = bass.Bass()
    xp = nc.declare_dram_parameter("xp", [_BPC, _NP], _BF16, isOutput=False)
    out = nc.declare_dram_parameter("out", [_BPC, _T, _WC], _BF16, isOutput=True)

    with (
        nc.sbuf_tensor([128, _FI], _BF16) as tin,
        nc.sbuf_tensor([128, _FO], _BF16) as tout,
        nc.Block() as block,
        nc.semaphore("l0") as l0,
        nc.semaphore("l1") as l1,
        nc.semaphore("l2") as l2,
        nc.semaphore("l3") as l3,
        nc.semaphore("es") as es,
        nc.semaphore("ss") as ss,
    ):
        lsem = [l0, l1, l2, l3]

        @block.gpsimd
        def _(e):
            for b in range(_BPC):
                e.dma_start(
                    out=AP(tin, b * _SEG, [[_FI, _PPB], [1, _SEG]]),
                    in_=AP(xp, b * _NP, [[_STEP, _PPB], [1, _SEG]]),
                ).then_inc(lsem[b], 16)
            for pair in range(_BPC // 2):
                e.wait_ge(es, 2 * (pair + 1))
                e.dma_start(
                    out=AP(
                        out,
                        pair * 2 * _TWC,
                        [[_RW, _PPB], [_TWC, 2], [1, _RW]],
                    ),
                    in_=AP(
                        tout,
                        pair * 2 * _RW,
                        [[_FO, _PPB], [_RW, 2], [1, _RW]],
                    ),
                ).then_inc(ss, 16)
            e.wait_ge(ss, 16 * (_BPC // 2))

        @block.vector
        def _(v):
            for b in range(_BPC):
                v.wait_ge(lsem[b], 16)
                v.tensor_copy(
                    out=AP(
                        tout, b * _RW, [[_FO, _PPB], [_WC, _RPP], [1, _WC]]
                    ).bitcast(_I32),
                    in_=AP(
                        tin, b * _SEG, [[_FI, _PPB], [_C, _RPP], [1, _WC]]
                    ).bitcast(_I32),
                ).then_inc(es, 1)

    _nc_cache = nc
    return nc


def _make_in_maps(x: np.ndarray) -> list[dict]:
    """x: [B, T, C] float32 -> per-core padded bf16 flat inputs."""
    xb = np.asarray(x, dtype=np.float32).astype(_NPBF16)
    xpad = np.zeros((_B, _NP), _NPBF16)
    xpad[:, _PAD : _PAD + _T * _C] = xb.reshape(_B, _T * _C)
    return [
        {"xp": np.ascontiguousarray(xpad[i * _BPC : (i + 1) * _BPC])}
        for i in range(_NCORES)
    ]


def _gather_out(results) -> np.ndarray:
    return np.concatenate(
        [np.asarray(r["out"]).astype(np.float32) for r in results], axis=0
    ).reshape(_B, _T, _WC)


def kernel(x: np.ndarray) -> np.ndarray:
    assert np.asarray(x).shape == (_B, _T, _C)
    nc = _build()
    res = run_bass_kernel_spmd(nc, _make_in_maps(x), list(range(_NCORES)))
    return _gather_out(res.results)


# revision 17
# speedup vs baseline: 1.1277x; 1.1277x over previous
"""Trainium2 Bass kernel for nn_CreateOverlappingWindows.

out[b, t, w*C + c] = x_padded[b, t + w, c]  (SAME zero padding, n_context=9)

Flattening (w, c) -> 494 contiguous values, each output row is a contiguous
494-element window of the zero-padded flattened input:
    out[b, t, :] = xpad_flat[b, t*C : t*C + W*C]

Strategy (memory-regime): bf16 end-to-end (tolerance 2e-2; bf16 keeps f32's
exponent range so rel err <= 2^-9 ~ 2e-3 everywhere).

The output write dominates (19x the input bytes). Small descriptors are
descriptor-rate-bound (HWDGE ~10.4ns/desc; 988 B window descriptors cap at
~95 GB/s per ring), so instead:
  * SWDGE-load the tiny padded input into SBUF: per batch, partition p of
    100 holds the 988-element slice xpad_flat[b, 520*p : 520*p + 988]
    (20 output rows + 468-element halo).
  * DVE expands windows in-SBUF into a dense tile (int32-viewed copies,
    ~2.7us/batch; overlapping reads are free on a compute engine).
  * Two fused SWDGE stores (2 batches each, 200 x 19.8 KB fully-contiguous
    descriptors) stream the dense tile out at ~300+ GB/s. 19.8 KB is the
    per-descriptor sweet spot (>~20 KB halves the rate); big fused DMAs
    amortize the ~0.45us/engine doorbell ramp.

Sharding: pure data parallel - batch 32 split 4-per-core across 8 cores.
"""

import sys

sys.path.insert(0, "/opt/trn_rl_repo")

import ml_dtypes
import numpy as np
from concourse import bass, mybir
from concourse.ap import AP
from concourse.bass_utils import run_bass_kernel_spmd

_BF16 = mybir.dt.bfloat16
_I32 = mybir.dt.int32
_NPBF16 = ml_dtypes.bfloat16

_NCORES = 8
_B, _T, _C = 32, 2000, 26
_NCTX = 9
_W = 2 * _NCTX + 1  # 19
_WC = _W * _C  # 494
_PAD = _NCTX * _C  # 234
_BPC = _B // _NCORES  # 4 batches per core
_NP = _T * _C + 2 * _PAD  # 52468 padded flat length per batch
_TWC = _T * _WC  # 988000

_PPB = 100  # partitions per batch
_RPP = 20  # output rows per partition  (100 * 20 = 2000)
_STEP = _RPP * _C  # 520: flat-input stride between partition slices
_SEG = _STEP + (_WC - _C)  # 988: slice length incl. 468-element halo
_FI = _BPC * _SEG  # free elems/partition, input tile
_RW = _RPP * _WC  # 9880: dense output elems/partition/batch
_FO = _BPC * _RW  # 39520: free elems/partition, output tile

_nc_cache = None


def _build():
    global _nc_cache
    if _nc_cache is not None:
        return _nc_cache
    nc = bass.Bass()
    xp = nc.declare_dram_parameter("xp", [_BPC, _NP], _BF16, isOutput=False)
    out = nc.declare_dram_parameter("out", [_BPC, _T, _WC], _BF16, isOutput=True)

    with (
        nc.sbuf_tensor([128, _FI], _BF16) as tin,
        nc.sbuf_tensor([128, _FO], _BF16) as tout,
        nc.Block() as block,
        nc.semaphore("l0") as l0,
        nc.semaphore("l1") as l1,
        nc.semaphore("l2") as l2,
        nc.semaphore("l3") as l3,
        nc.semaphore("es") as es,
        nc.semaphore("ss") as ss,
    ):
        lsem = [l0, l1, l2, l3]

        def store(e, b):
            return e.dma_start(
                out=AP(out, b * _TWC, [[_RW, _PPB], [1, _RW]]),
                in_=AP(tout, b * _RW, [[_FO, _PPB], [1, _RW]]),
            )

        @block.gpsimd
        def _(e):
            for b in range(_BPC):
                e.dma_start(
                    out=AP(tin, b * _SEG, [[_FI, _PPB], [1, _SEG]]),
                    in_=AP(xp, b * _NP, [[_STEP, _PPB], [1, _SEG]]),
                ).then_inc(lsem[b], 16)
            for b in (2, 3):
                e.wait_ge(es, b + 1)
                store(e, b).then_inc(ss, 16)
            e.wait_ge(ss, 64)

        @block.sync
        def _(e):
            e.wait_ge(es, 1)
            store(e, 0).then_inc(ss, 16)
            e.wait_ge(ss, 64)

        @block.scalar
        def _(e):
            e.wait_ge(es, 2)
            store(e, 1).then_inc(ss, 16)
            e.wait_ge(ss, 64)

        @block.vector
        def _(v):
            for b in range(_BPC):
                v.wait_ge(lsem[b], 16)
                v.tensor_copy(
                    out=AP(
                        tout, b * _RW, [[_FO, _PPB], [_WC, _RPP], [1, _WC]]
                    ).bitcast(_I32),
                    in_=AP(
                        tin, b * _SEG, [[_FI, _PPB], [_C, _RPP], [1, _WC]]
                    ).bitcast(_I32),
                ).then_inc(es, 1)

    _nc_cache = nc
    return nc


def _make_in_maps(x: np.ndarray) -> list[dict]:
    """x: [B, T, C] float32 -> per-core padded bf16 flat inputs."""
    xb = np.asarray(x, dtype=np.float32).astype(_NPBF16)
    xpad = np.zeros((_B, _NP), _NPBF16)
    xpad[:, _PAD : _PAD + _T * _C] = xb.reshape(_B, _T * _C)
    return [
        {"xp": np.ascontiguousarray(xpad[i * _BPC : (i + 1) * _BPC])}
        for i in range(_NCORES)
    ]


def _gather_out(results) -> np.ndarray:
    return np.concatenate(
        [np.asarray(r["out"]).astype(np.float32) for r in results], axis=0
    ).reshape(_B, _T, _WC)


def kernel(x: np.ndarray) -> np.ndarray:
    assert np.asarray(x).shape == (_B, _T, _C)
    nc = _build()
    res = run_bass_kernel_spmd(nc, _make_in_maps(x), list(range(_NCORES)))
    return _gather_out(res.results)


# revision 18
# speedup vs baseline: 1.4242x; 1.2630x over previous
"""Trainium2 Bass kernel for nn_CreateOverlappingWindows.

out[b, t, w*C + c] = x_padded[b, t + w, c]  (SAME zero padding, n_context=9)

Flattening (w, c) -> 494 contiguous values, each output row is a contiguous
494-element window of the zero-padded flattened input:
    out[b, t, :] = xpad_flat[b, t*C : t*C + W*C]

Strategy (memory-regime): bf16 end-to-end (tolerance 2e-2; bf16 keeps f32's
exponent range so rel err <= 2^-9 ~ 2e-3 everywhere).

SBUF-involved DMA descriptors start at a shared ~85ns/descriptor pace
(regardless of queue), so SBUF->DRAM throughput == descriptor_size / 85ns.
DRAM->DRAM descriptors pace at the HWDGE ring rate (~10.4ns/desc).

  * Batches 0-2 (SBUF path): SWDGE loads the padded input into SBUF
    (80 partitions x 1118-elem overlapping slices = 25 output rows + halo),
    DVE expands windows into a dense tile (int32-viewed copies,
    ~3.3us/batch), SWDGE streams out 80 x 24.7 KB contiguous descriptors
    per batch.
  * Batch 3 (direct path): two half-batch window-gathers DRAM->DRAM
    (1000 x 988 B descs each) on the sync and scalar HWDGE rings, gated
    on the loads so the tiny loads win the engines first.

Sharding: pure data parallel - batch 32 split 4-per-core across 8 cores.
"""

import sys

sys.path.insert(0, "/opt/trn_rl_repo")

import ml_dtypes
import numpy as np
from concourse import bass, mybir
from concourse.ap import AP
from concourse.bass_utils import run_bass_kernel_spmd

_BF16 = mybir.dt.bfloat16
_I32 = mybir.dt.int32
_NPBF16 = ml_dtypes.bfloat16

_NCORES = 8
_B, _T, _C = 32, 2000, 26
_NCTX = 9
_W = 2 * _NCTX + 1  # 19
_WC = _W * _C  # 494
_PAD = _NCTX * _C  # 234
_BPC = _B // _NCORES  # 4 batches per core
_NP = _T * _C + 2 * _PAD  # 52468 padded flat length per batch
_TWC = _T * _WC  # 988000

_NSB = 3  # batches on the SBUF/SWDGE path; the last goes direct HWDGE

_PPB = 80  # partitions per batch
_RPP = 25  # output rows per partition  (80 * 25 = 2000)
_STEP = _RPP * _C  # 650: flat-input stride between partition slices
_SEG = _STEP + (_WC - _C)  # 1118: slice length incl. 468-element halo
_FI = _NSB * _SEG  # free elems/partition, input tile
_RW = _RPP * _WC  # 12350: dense output elems/partition/batch
_FO = _NSB * _RW  # free elems/partition, output tile

_nc_cache = None


def _build():
    global _nc_cache
    if _nc_cache is not None:
        return _nc_cache
    nc = bass.Bass()
    xp = nc.declare_dram_parameter("xp", [_BPC, _NP], _BF16, isOutput=False)
    out = nc.declare_dram_parameter("out", [_BPC, _T, _WC], _BF16, isOutput=True)

    with (
        nc.sbuf_tensor([128, _FI], _BF16) as tin,
        nc.sbuf_tensor([128, _FO], _BF16) as tout,
        nc.Block() as block,
        nc.semaphore("l0") as l0,
        nc.semaphore("l1") as l1,
        nc.semaphore("l2") as l2,
        nc.semaphore("es") as es,
        nc.semaphore("ss") as ss,
        nc.semaphore("hs") as hs,
    ):
        lsem = [l0, l1, l2]

        @block.gpsimd
        def _(e):
            for b in range(_NSB):
                e.dma_start(
                    out=AP(tin, b * _SEG, [[_FI, _PPB], [1, _SEG]]),
                    in_=AP(xp, b * _NP, [[_STEP, _PPB], [1, _SEG]]),
                ).then_inc(lsem[b], 16)
            for b in range(_NSB):
                e.wait_ge(es, b + 1)
                e.dma_start(
                    out=AP(out, b * _TWC, [[_RW, _PPB], [1, _RW]]),
                    in_=AP(tout, b * _RW, [[_FO, _PPB], [1, _RW]]),
                ).then_inc(ss, 16)
            e.wait_ge(ss, 16 * _NSB)

        @block.vector
        def _(v):
            for b in range(_NSB):
                v.wait_ge(lsem[b], 16)
                v.tensor_copy(
                    out=AP(
                        tout, b * _RW, [[_FO, _PPB], [_WC, _RPP], [1, _WC]]
                    ).bitcast(_I32),
                    in_=AP(
                        tin, b * _SEG, [[_FI, _PPB], [_C, _RPP], [1, _WC]]
                    ).bitcast(_I32),
                ).then_inc(es, 1)

        _HT = _T // 2  # 1000 rows per half-gather

        @block.sync
        def _(e):
            for s in lsem:
                e.wait_ge(s, 16)
            b = _NSB  # batch 3 rows 0-999: direct DRAM->DRAM gather
            e.dma_start(
                out=AP(out, b * _TWC, [[_WC, _HT], [1, _WC]]),
                in_=AP(xp, b * _NP, [[_C, _HT], [1, _WC]]),
            ).then_inc(hs, 16)
            e.wait_ge(hs, 32)

        @block.scalar
        def _(e):
            for s in lsem:
                e.wait_ge(s, 16)
            b = _NSB  # batch 3 rows 1000-1999
            e.dma_start(
                out=AP(out, b * _TWC + _HT * _WC, [[_WC, _HT], [1, _WC]]),
                in_=AP(xp, b * _NP + _HT * _C, [[_C, _HT], [1, _WC]]),
            ).then_inc(hs, 16)
            e.wait_ge(hs, 32)

    _nc_cache = nc
    return nc


def _make_in_maps(x: np.ndarray) -> list[dict]:
    """x: [B, T, C] float32 -> per-core padded bf16 flat inputs."""
    xb = np.asarray(x, dtype=np.float32).astype(_NPBF16)
    xpad = np.zeros((_B, _NP), _NPBF16)
    xpad[:, _PAD : _PAD + _T * _C] = xb.reshape(_B, _T * _C)
    return [
        {"xp": np.ascontiguousarray(xpad[i * _BPC : (i + 1) * _BPC])}
        for i in range(_NCORES)
    ]


def _gather_out(results) -> np.ndarray:
    return np.concatenate(
        [np.asarray(r["out"]).astype(np.float32) for r in results], axis=0
    ).reshape(_B, _T, _WC)


def kernel(x: np.ndarray) -> np.ndarray:
    assert np.asarray(x).shape == (_B, _T, _C)
    nc = _build()
    res = run_bass_kernel_spmd(nc, _make_in_maps(x), list(range(_NCORES)))
    return _gather_out(res.results)
